# revision 29
# baseline (speedup 1.0000x reference)
"""Attention-pooling kernel for Trainium2 (Bass/Tile), 8-core data parallel.

Problem: for each batch item b (256 total):
    scores = E_b @ w_att            # [512]
    attn   = softmax(scores)        # [512]
    pooled = attn @ E_b             # [768]
    out_b  = sigmoid(pooled @ w_pred + b_pred)

Sharding: batch 256 -> 8 cores x 32 items. Weights replicated.

Per-core design. Embeddings are cast to bf16 on the host, so the HBM
stream is 24 MiB/core -> ~73 us at the 360 GB/s DMA roofline; measured
isolated engine rates (differential NEFF timing): DMA 73, DVE ~66,
ScalarE ~75-125, PE ~26 us per execution. Per item:
  - E_b loaded once as [128, 4*768] bf16 with s = 4p + c; each partition is
    a single contiguous 6 KiB run in HBM -> full-rate DMA.
  - items processed in QUADS: one DMA and one DVE tensor_tensor (16-bit 2x
    mode) per quad writes the bf16 product tile P = E * w_att (the PE's
    moving operand) - grouping amortizes per-instruction fixed costs.
  - score sums (16 per quad): 10 via ScalarE activation-Copy accum_out (the
    elementwise out goes to a shared dummy so the PE has no false dep on
    P), 6 via DVE tensor_reduce (chunk 3 for all four items in one reduce,
    chunk 2 for two). Balanced ~3 us/item on each engine, under the
    4.4 us/item DMA budget. (tensor_tensor_reduce would
    fuse multiply+reduce in one DVE op, but InstTensorTensorReduce wedges
    the exec unit on this hardware - verified by bisection; InstPool
    fails walrus codegen.)
  - u = exp(sc) on ScalarE writes bf16 into column i of a zeroed
    [128, 4, n] tile (zero-fill on the otherwise-idle GPSIMD). Softmax
    max-subtraction skipped: scores ~ N(0,1), exp is safe in f32.
  - pooled via PE in bf16 (1 cycle/row): per (item, chunk) two matmuls over
    P_c (cols 0:512 / 512:768, split at the PSUM bank boundary), lhsT = the
    n-wide u column block (only col i nonzero), accumulating item i into
    PSUM partition i of one persistent [n, 1024] f32 tile. This computes
    pooledP_d = w_att_d * pooled_d; the host folds w_pred_d / w_att_d into
    the shipped weight vector so the final dot recovers pooled @ w_pred
    (bf16 rounding of P scales with w_att_d, so the division does not
    amplify error). U = sum_{p,c} u is computed by one extra 1-column
    matmul per (item, chunk) (rhs = ones[128,1]) into PSUM col 768 -
    essentially free on the PE, which runs at ~30% occupancy.
  - single batched tail: DVE mult + ScalarE accum for the [n, 768] dot
    (read straight from PSUM), reciprocal of U, fused sigmoid(dot/U + b),
    one tiny DMA of the [n] outputs.
"""

import os
import sys

import numpy as np

_REPO = "/opt/trn_rl_repo"
if _REPO not in sys.path:
    sys.path.insert(0, _REPO)

from contextlib import ExitStack

import concourse.bass as bass
import concourse.tile as tile
from concourse import bacc, mybir
from concourse.bass_utils import run_bass_kernel_spmd

N_CORES = 8
B = 256
S = 512
D = 768
PER_CORE = B // N_CORES  # 32
C = S // 128  # 4 s-chunks per item
WCAT = 2 * D + 1  # w_att | w_pred/w_att | b_pred

f32 = mybir.dt.float32
bf16 = mybir.dt.bfloat16
Alu = mybir.AluOpType
Act = mybir.ActivationFunctionType


def make_wcat(w_att, w_pred, b_pred):
    w_att = np.asarray(w_att, np.float64).reshape(D)
    w_pred = np.asarray(w_pred, np.float64).reshape(D)
    b = float(np.asarray(b_pred).reshape(()))
    # the device multiplies by bf16(w_att); divide by the same rounded value
    import ml_dtypes

    wa16 = w_att.astype(ml_dtypes.bfloat16).astype(np.float64)
    rr = w_pred / wa16  # pooledP_d = wa16_d * pooled_d absorbs the division
    return np.concatenate([w_att, rr, [b]]).astype(np.float32).reshape(1, WCAT)


def prep_emb(embeddings):
    import ml_dtypes

    return np.ascontiguousarray(embeddings).astype(ml_dtypes.bfloat16)


def build_kernel(n_items: int = PER_CORE, reps: int = 1):
    """reps > 1 builds a timing variant: the whole pipeline (including the
    HBM streaming) repeats back-to-back inside one NEFF execution, so
    steady-state wall time / reps amortizes host dispatch overhead."""
    assert n_items <= 32  # PSUM partitions 0:n, stationary <= 32 cols
    nc = bacc.Bacc(None, target_bir_lowering=False)

    emb = nc.dram_tensor("emb", [n_items, S, D], bf16, kind="ExternalInput")
    wcat = nc.dram_tensor("wcat", [1, WCAT], f32, kind="ExternalInput")
    out = nc.dram_tensor("out", [n_items], f32, kind="ExternalOutput")

    with tile.TileContext(nc) as tc:
        with ExitStack() as ctx:
            const = ctx.enter_context(tc.tile_pool(name="const", bufs=1))
            e_pool = ctx.enter_context(tc.tile_pool(name="e", bufs=3))
            p_pool = ctx.enter_context(tc.tile_pool(name="p16", bufs=3))
            u_pool = ctx.enter_context(tc.tile_pool(name="u", bufs=8))
            sc_pool = ctx.enter_context(tc.tile_pool(name="sc", bufs=3))
            fin_pool = ctx.enter_context(tc.tile_pool(name="fin", bufs=2))
            ps_pool = ctx.enter_context(tc.tile_pool(name="ps", bufs=1, space="PSUM"))

            # ---- constants
            wrep = const.tile([128, WCAT], f32)
            nc.gpsimd.dma_start(
                out=wrep[:, :], in_=wcat[0:1, :].broadcast_to([128, WCAT])
            )
            wrep16 = const.tile([128, D], bf16)
            nc.scalar.copy(out=wrep16[:, :], in_=wrep[:, 0:D])
            ones1 = const.tile([128, 1], bf16)
            nc.vector.memset(ones1[:, :], 1.0)
            dummy = const.tile([128, D], bf16)  # accum-copies' discard target

            # one persistent PSUM accumulator: partition i = item i,
            # cols 0:768 = pooledP, col 768 = U
            ps = ps_pool.tile([n_items, 1024], f32)

            assert n_items % 4 == 0
            G = 4  # items per DMA/multiply group (amortizes fixed costs)
            for rep, t in ((r, j) for r in range(reps) for j in range(n_items // G)):
                i0 = G * t
                et2 = e_pool.tile([128, G, C * D], bf16, tag="et")
                src = emb[i0 : i0 + G, :, :].rearrange(
                    "o (p c) d -> p o c d", p=128, c=C
                )
                nc.sync.dma_start(out=et2[:, :, :], in_=src)

                p2 = p_pool.tile([128, G, C * D], bf16, tag="p16")
                nc.vector.tensor_tensor(
                    out=p2[:, :, :],
                    in0=et2[:, :, :],
                    in1=wrep16[:, :].unsqueeze(1).broadcast_to([128, G * C, D]),
                    op=Alu.mult,
                )

                # score sums, 16 per quad: 10 on ScalarE (accum-copy to a
                # dummy, so the PE has no false dep on P), 6 on DVE (chunk 3
                # for all four items in one reduce, chunk 2 for items 2,3 in
                # another). Balanced ~3 us/item on each engine.
                sc2 = sc_pool.tile([128, G, C], f32, tag="sc")
                nc.vector.tensor_reduce(
                    out=sc2[:, :, 3],
                    in_=p2[:, :, 3 * D : 4 * D],
                    axis=mybir.AxisListType.X,
                    op=Alu.add,
                )
                nc.vector.tensor_reduce(
                    out=sc2[:, 2:4, 2],
                    in_=p2[:, 2:4, 2 * D : 3 * D],
                    axis=mybir.AxisListType.X,
                    op=Alu.add,
                )
                # interleave each item's accums with its exp so the PE can
                # start on item j while items j+1.. are still reducing --
                # shortens the P-tile lifetime so DMA never waits on buffers
                uts = []
                for j in range(G):
                    for c in ((0, 1) if j >= 2 else (0, 1, 2)):
                        nc.scalar.activation(
                            out=dummy[:, :],
                            in_=p2[:, j, c * D : (c + 1) * D],
                            func=Act.Copy,
                            accum_out=sc2[:, j, c : c + 1],
                        )
                    ut = u_pool.tile([128, C, n_items], bf16, tag="ut")
                    nc.gpsimd.memset(ut[:, :, :], 0.0)
                    nc.scalar.activation(
                        out=ut[:, 0:C, i0 + j : i0 + j + 1],
                        in_=sc2[:, j, :],
                        func=Act.Exp,
                    )
                    uts.append(ut)

                for j in range(G):
                    i = i0 + j
                    ut = uts[j]

                    # PSUM bank0 = cols 0:512, bank1 = cols 512:1024. One
                    # accumulation group per bank; bank1's last touch is the
                    # final U ones-matmul.
                    last = i == n_items - 1
                    for c in range(C):
                        nc.tensor.matmul(
                            out=ps[0:n_items, 0:512],
                            lhsT=ut[:, c, :],
                            rhs=p2[:, j, c * D : c * D + 512],
                            start=(i == 0 and c == 0),
                            stop=(last and c == C - 1),
                        )
                        nc.tensor.matmul(
                            out=ps[0:n_items, 512:768],
                            lhsT=ut[:, c, :],
                            rhs=p2[:, j, c * D + 512 : (c + 1) * D],
                            start=(i == 0 and c == 0),
                            stop=False,
                        )
                        nc.tensor.matmul(
                            out=ps[0:n_items, D : D + 1],
                            lhsT=ut[:, c, :],
                            rhs=ones1[:, :],
                            start=False,
                            stop=(last and c == C - 1),
                        )

                if i0 + G != n_items:
                    continue
                # ---- batched tail over all n items (once per rep)
                scrt = fin_pool.tile([n_items, D], f32, tag="scrt")
                nc.vector.tensor_tensor(
                    out=scrt[:, :],
                    in0=ps[0:n_items, 0:D],
                    in1=wrep[0:n_items, D : 2 * D],
                    op=Alu.mult,
                )
                dz = fin_pool.tile([n_items, 1], f32, tag="dz")
                nc.scalar.activation(
                    out=scrt[:, :], in_=scrt[:, :], func=Act.Copy, accum_out=dz[:, :]
                )
                rU = fin_pool.tile([n_items, 1], f32, tag="rU")
                nc.vector.reciprocal(out=rU[:, :], in_=ps[0:n_items, D : D + 1])
                t = fin_pool.tile([n_items, 1], f32, tag="t")
                nc.vector.tensor_tensor(
                    out=t[:, :], in0=dz[:, :], in1=rU[:, :], op=Alu.mult
                )
                sg = fin_pool.tile([n_items, 1], f32, tag="sg")
                nc.scalar.activation(
                    out=sg[:, :],
                    in_=t[:, :],
                    func=Act.Sigmoid,
                    bias=wrep[0:n_items, 2 * D : 2 * D + 1],
                    scale=1.0,
                )
                nc.sync.dma_start(out=out[0:n_items], in_=sg[:, :])

    nc.compile()
    return nc


_NC_CACHE: dict[int, object] = {}


def _get_nc(n_items: int = PER_CORE):
    if n_items not in _NC_CACHE:
        _NC_CACHE[n_items] = build_kernel(n_items)
    return _NC_CACHE[n_items]


def make_runner(nc, in_maps):
    """Replicate bass2jax.run_bass_via_pjrt's multi-core path without output
    donation, returning (jitted_fn, device_args, out_names) so executions can
    be timed with inputs resident on device."""
    import jax
    import jax.numpy as jnp
    from jax.sharding import Mesh, PartitionSpec
    try:
        from jax.experimental.shard_map import shard_map
    except ImportError:
        from jax.shard_map import shard_map

    from concourse import bass2jax as b2j
    from concourse import mybir as mb

    b2j.install_neuronx_cc_hook()

    partition_name = nc.partition_id_tensor.name if nc.partition_id_tensor else None
    in_names, out_names, out_avals, zero_outs = [], [], [], []
    for alloc in nc.m.functions[0].allocations:
        if not isinstance(alloc, mb.MemoryLocationSet):
            continue
        name = alloc.memorylocations[0].name
        if alloc.kind == "ExternalInput":
            if name != partition_name:
                in_names.append(name)
        elif alloc.kind == "ExternalOutput":
            out_names.append(name)
            shape = tuple(alloc.tensor_shape)
            dtype = mb.dt.np(alloc.dtype)
            out_avals.append(jax.core.ShapedArray(shape, dtype))
            zero_outs.append(np.zeros(shape, dtype))
    n_params = len(in_names)
    all_in_names = list(in_names) + list(out_names)
    if partition_name is not None:
        all_in_names.append(partition_name)

    def _body(*args):
        operands = list(args)
        if partition_name is not None:
            operands.append(b2j.partition_id_tensor())
        outs = b2j._bass_exec_p.bind(
            *operands,
            out_avals=tuple(out_avals),
            in_names=tuple(all_in_names),
            out_names=tuple(out_names),
            lowering_input_output_aliases=(),
            sim_require_finite=True,
            sim_require_nnan=True,
            nc=nc,
        )
        return tuple(outs)

    n_cores = len(in_maps)
    devices = jax.devices()[:n_cores]
    mesh = Mesh(np.asarray(devices), ("core",))
    in_specs = (PartitionSpec("core"),) * (n_params + len(out_names))
    out_specs = (PartitionSpec("core"),) * len(out_names)
    fn = jax.jit(
        shard_map(
            _body, mesh=mesh, in_specs=in_specs, out_specs=out_specs, check_rep=False
        ),
        keep_unused=True,
    )

    per_core = [[np.asarray(m[name]) for name in in_names] for m in in_maps]
    concat_in = [
        np.concatenate([per_core[c][i] for c in range(n_cores)], axis=0)
        for i in range(n_params)
    ]
    concat_zeros = [
        np.zeros((n_cores * z.shape[0], *z.shape[1:]), z.dtype) for z in zero_outs
    ]
    sharding = jax.sharding.NamedSharding(mesh, PartitionSpec("core"))
    args = [jax.device_put(a, sharding) for a in concat_in + concat_zeros]
    return fn, args, out_names, out_avals


def kernel(embeddings, w_att, w_pred, b_pred, **run_kwargs):
    embeddings = prep_emb(embeddings)
    wcat = make_wcat(w_att, w_pred, b_pred)

    nc = _get_nc(PER_CORE)
    in_maps = [
        {
            "emb": embeddings[i * PER_CORE : (i + 1) * PER_CORE],
            "wcat": wcat,
        }
        for i in range(N_CORES)
    ]
    res = run_bass_kernel_spmd(nc, in_maps, core_ids=list(range(N_CORES)), **run_kwargs)
    outs = [res.results[i]["out"].reshape(-1)[:PER_CORE] for i in range(N_CORES)]
    full = np.concatenate(outs).astype(np.float32)
    if run_kwargs:
        return full, res
    return full


# revision 30
# speedup vs baseline: 1.3232x; 1.3232x over previous
"""Attention-pooling kernel for Trainium2 (Bass/Tile), 8-core data parallel.

Problem: for each batch item b (256 total):
    scores = E_b @ w_att            # [512]
    attn   = softmax(scores)        # [512]
    pooled = attn @ E_b             # [768]
    out_b  = sigmoid(pooled @ w_pred + b_pred)

Sharding: batch 256 -> 8 cores x 32 items. Weights replicated.

Per-core design. Embeddings are cast to bf16 on the host, so the HBM
stream is 24 MiB/core -> ~73 us at the 360 GB/s DMA roofline; measured
isolated engine rates (differential NEFF timing): DMA 73, DVE ~66,
ScalarE ~75-125, PE ~26 us per execution. Per item:
  - E_b loaded once as [128, 4*768] bf16 with s = 4p + c; each partition is
    a single contiguous 6 KiB run in HBM -> full-rate DMA.
  - items processed in QUADS: one DMA and one DVE tensor_tensor (16-bit 2x
    mode) per quad writes the bf16 product tile P = E * w_att (the PE's
    moving operand) - grouping amortizes per-instruction fixed costs.
  - score sums (16 per quad): 10 via ScalarE activation-Copy accum_out (the
    elementwise out goes to a shared dummy so the PE has no false dep on
    P), 6 via DVE tensor_reduce (chunk 3 for all four items in one reduce,
    chunk 2 for two). Balanced ~3 us/item on each engine, under the
    4.4 us/item DMA budget. (tensor_tensor_reduce would
    fuse multiply+reduce in one DVE op, but InstTensorTensorReduce wedges
    the exec unit on this hardware - verified by bisection; InstPool
    fails walrus codegen.)
  - u = exp(sc) on ScalarE writes bf16 into column i of a zeroed
    [128, 4, n] tile (zero-fill on the otherwise-idle GPSIMD). Softmax
    max-subtraction skipped: scores ~ N(0,1), exp is safe in f32.
  - pooled via PE in bf16 (1 cycle/row): per (item, chunk) two matmuls over
    P_c (cols 0:512 / 512:768, split at the PSUM bank boundary), lhsT = the
    n-wide u column block (only col i nonzero), accumulating item i into
    PSUM partition i of one persistent [n, 1024] f32 tile. This computes
    pooledP_d = w_att_d * pooled_d; the host folds w_pred_d / w_att_d into
    the shipped weight vector so the final dot recovers pooled @ w_pred
    (bf16 rounding of P scales with w_att_d, so the division does not
    amplify error). U = sum_{p,c} u is computed by one extra 1-column
    matmul per (item, chunk) (rhs = ones[128,1]) into PSUM col 768 -
    essentially free on the PE, which runs at ~30% occupancy.
  - single batched tail: DVE mult + ScalarE accum for the [n, 768] dot
    (read straight from PSUM), reciprocal of U, fused sigmoid(dot/U + b),
    one tiny DMA of the [n] outputs.
"""

import os
import sys

import numpy as np

_REPO = "/opt/trn_rl_repo"
if _REPO not in sys.path:
    sys.path.insert(0, _REPO)

from contextlib import ExitStack

import concourse.bass as bass
import concourse.tile as tile
from concourse import bacc, mybir
from concourse.bass_utils import run_bass_kernel_spmd

N_CORES = 8
B = 256
S = 512
D = 768
PER_CORE = B // N_CORES  # 32
C = S // 128  # 4 s-chunks per item
WCAT = 2 * D + 1  # w_att | w_pred/w_att | b_pred

f32 = mybir.dt.float32
bf16 = mybir.dt.bfloat16
Alu = mybir.AluOpType
Act = mybir.ActivationFunctionType


def make_wcat(w_att, w_pred, b_pred):
    w_att = np.asarray(w_att, np.float64).reshape(D)
    w_pred = np.asarray(w_pred, np.float64).reshape(D)
    b = float(np.asarray(b_pred).reshape(()))
    # the device multiplies by bf16(w_att); divide by the same rounded value
    import ml_dtypes

    wa16 = w_att.astype(ml_dtypes.bfloat16).astype(np.float64)
    rr = w_pred / wa16  # pooledP_d = wa16_d * pooled_d absorbs the division
    return np.concatenate([w_att, rr, [b]]).astype(np.float32).reshape(1, WCAT)


def prep_emb(embeddings):
    import ml_dtypes

    return np.ascontiguousarray(embeddings).astype(ml_dtypes.bfloat16)


def build_kernel(n_items: int = PER_CORE, reps: int = 1):
    """reps > 1 builds a timing variant: the whole pipeline (including the
    HBM streaming) repeats back-to-back inside one NEFF execution, so
    steady-state wall time / reps amortizes host dispatch overhead."""
    assert n_items <= 32  # PSUM partitions 0:n, stationary <= 32 cols
    nc = bacc.Bacc(None, target_bir_lowering=False)

    emb = nc.dram_tensor("emb", [n_items, S, D], bf16, kind="ExternalInput")
    wcat = nc.dram_tensor("wcat", [1, WCAT], f32, kind="ExternalInput")
    out = nc.dram_tensor("out", [n_items], f32, kind="ExternalOutput")

    with tile.TileContext(nc) as tc:
        with ExitStack() as ctx:
            const = ctx.enter_context(tc.tile_pool(name="const", bufs=1))
            e_pool = ctx.enter_context(tc.tile_pool(name="e", bufs=3))
            p_pool = ctx.enter_context(tc.tile_pool(name="p16", bufs=3))
            u_pool = ctx.enter_context(tc.tile_pool(name="u", bufs=3))
            sc_pool = ctx.enter_context(tc.tile_pool(name="sc", bufs=3))
            fin_pool = ctx.enter_context(tc.tile_pool(name="fin", bufs=2))
            ps_pool = ctx.enter_context(tc.tile_pool(name="ps", bufs=1, space="PSUM"))

            # ---- constants
            wrep = const.tile([128, WCAT], f32)
            nc.gpsimd.dma_start(
                out=wrep[:, :], in_=wcat[0:1, :].broadcast_to([128, WCAT])
            )
            wrep16 = const.tile([128, D], bf16)
            nc.scalar.copy(out=wrep16[:, :], in_=wrep[:, 0:D])
            ones1 = const.tile([128, 1], bf16)
            nc.vector.memset(ones1[:, :], 1.0)
            dummy = const.tile([128, D], bf16)  # accum-copies' discard target

            # one persistent PSUM accumulator: partition i = item i,
            # cols 0:768 = pooledP, col 768 = U
            ps = ps_pool.tile([n_items, 1024], f32)

            assert n_items % 4 == 0
            G = 4  # items per DMA/multiply group (amortizes fixed costs)
            for rep, t in ((r, j) for r in range(reps) for j in range(n_items // G)):
                i0 = G * t
                et2 = e_pool.tile([128, G, C * D], bf16, tag="et")
                src = emb[i0 : i0 + G, :, :].rearrange(
                    "o (p c) d -> p o c d", p=128, c=C
                )
                nc.sync.dma_start(out=et2[:, :, :], in_=src)

                p2 = p_pool.tile([128, G, C * D], bf16, tag="p16")
                nc.vector.tensor_tensor(
                    out=p2[:, :, :],
                    in0=et2[:, :, :],
                    in1=wrep16[:, :].unsqueeze(1).broadcast_to([128, G * C, D]),
                    op=Alu.mult,
                )

                # score sums, 16 per quad: 10 on ScalarE (accum-copy to a
                # dummy, so the PE has no false dep on P), 6 on DVE (chunk 3
                # for all four items in one reduce, chunk 2 for items 2,3 in
                # another). Balanced ~3 us/item on each engine.
                sc2 = sc_pool.tile([128, G, C], f32, tag="sc")
                for j in range(G):
                    for c in ((0, 1) if j >= 2 else (0, 1, 2)):
                        nc.scalar.activation(
                            out=dummy[:, :],
                            in_=p2[:, j, c * D : (c + 1) * D],
                            func=Act.Copy,
                            accum_out=sc2[:, j, c : c + 1],
                        )
                nc.vector.tensor_reduce(
                    out=sc2[:, :, 3],
                    in_=p2[:, :, 3 * D : 4 * D],
                    axis=mybir.AxisListType.X,
                    op=Alu.add,
                )
                nc.vector.tensor_reduce(
                    out=sc2[:, 2:4, 2],
                    in_=p2[:, 2:4, 2 * D : 3 * D],
                    axis=mybir.AxisListType.X,
                    op=Alu.add,
                )

                for j in range(G):
                    i = i0 + j
                    ut = u_pool.tile([128, C, n_items], bf16, tag="ut")
                    nc.gpsimd.memset(ut[:, :, :], 0.0)
                    nc.scalar.activation(
                        out=ut[:, 0:C, i : i + 1],
                        in_=sc2[:, j, :],
                        func=Act.Exp,
                    )

                    # PSUM bank0 = cols 0:512, bank1 = cols 512:1024. One
                    # accumulation group per bank; bank1's last touch is the
                    # final U ones-matmul.
                    last = i == n_items - 1
                    for c in range(C):
                        nc.tensor.matmul(
                            out=ps[0:n_items, 0:512],
                            lhsT=ut[:, c, :],
                            rhs=p2[:, j, c * D : c * D + 512],
                            start=(i == 0 and c == 0),
                            stop=(last and c == C - 1),
                        )
                        nc.tensor.matmul(
                            out=ps[0:n_items, 512:768],
                            lhsT=ut[:, c, :],
                            rhs=p2[:, j, c * D + 512 : (c + 1) * D],
                            start=(i == 0 and c == 0),
                            stop=False,
                        )
                        nc.tensor.matmul(
                            out=ps[0:n_items, D : D + 1],
                            lhsT=ut[:, c, :],
                            rhs=ones1[:, :],
                            start=False,
                            stop=(last and c == C - 1),
                        )

                if i0 + G != n_items:
                    continue
                # ---- batched tail over all n items (once per rep)
                scrt = fin_pool.tile([n_items, D], f32, tag="scrt")
                nc.vector.tensor_tensor(
                    out=scrt[:, :],
                    in0=ps[0:n_items, 0:D],
                    in1=wrep[0:n_items, D : 2 * D],
                    op=Alu.mult,
                )
                dz = fin_pool.tile([n_items, 1], f32, tag="dz")
                nc.scalar.activation(
                    out=scrt[:, :], in_=scrt[:, :], func=Act.Copy, accum_out=dz[:, :]
                )
                rU = fin_pool.tile([n_items, 1], f32, tag="rU")
                nc.vector.reciprocal(out=rU[:, :], in_=ps[0:n_items, D : D + 1])
                t = fin_pool.tile([n_items, 1], f32, tag="t")
                nc.vector.tensor_tensor(
                    out=t[:, :], in0=dz[:, :], in1=rU[:, :], op=Alu.mult
                )
                sg = fin_pool.tile([n_items, 1], f32, tag="sg")
                nc.scalar.activation(
                    out=sg[:, :],
                    in_=t[:, :],
                    func=Act.Sigmoid,
                    bias=wrep[0:n_items, 2 * D : 2 * D + 1],
                    scale=1.0,
                )
                nc.sync.dma_start(out=out[0:n_items], in_=sg[:, :])

    nc.compile()
    return nc


_NC_CACHE: dict[int, object] = {}


def _get_nc(n_items: int = PER_CORE):
    if n_items not in _NC_CACHE:
        _NC_CACHE[n_items] = build_kernel(n_items)
    return _NC_CACHE[n_items]


def make_runner(nc, in_maps):
    """Replicate bass2jax.run_bass_via_pjrt's multi-core path without output
    donation, returning (jitted_fn, device_args, out_names) so executions can
    be timed with inputs resident on device."""
    import jax
    import jax.numpy as jnp
    from jax.sharding import Mesh, PartitionSpec
    try:
        from jax.experimental.shard_map import shard_map
    except ImportError:
        from jax.shard_map import shard_map

    from concourse import bass2jax as b2j
    from concourse import mybir as mb

    b2j.install_neuronx_cc_hook()

    partition_name = nc.partition_id_tensor.name if nc.partition_id_tensor else None
    in_names, out_names, out_avals, zero_outs = [], [], [], []
    for alloc in nc.m.functions[0].allocations:
        if not isinstance(alloc, mb.MemoryLocationSet):
            continue
        name = alloc.memorylocations[0].name
        if alloc.kind == "ExternalInput":
            if name != partition_name:
                in_names.append(name)
        elif alloc.kind == "ExternalOutput":
            out_names.append(name)
            shape = tuple(alloc.tensor_shape)
            dtype = mb.dt.np(alloc.dtype)
            out_avals.append(jax.core.ShapedArray(shape, dtype))
            zero_outs.append(np.zeros(shape, dtype))
    n_params = len(in_names)
    all_in_names = list(in_names) + list(out_names)
    if partition_name is not None:
        all_in_names.append(partition_name)

    def _body(*args):
        operands = list(args)
        if partition_name is not None:
            operands.append(b2j.partition_id_tensor())
        outs = b2j._bass_exec_p.bind(
            *operands,
            out_avals=tuple(out_avals),
            in_names=tuple(all_in_names),
            out_names=tuple(out_names),
            lowering_input_output_aliases=(),
            sim_require_finite=True,
            sim_require_nnan=True,
            nc=nc,
        )
        return tuple(outs)

    n_cores = len(in_maps)
    devices = jax.devices()[:n_cores]
    mesh = Mesh(np.asarray(devices), ("core",))
    in_specs = (PartitionSpec("core"),) * (n_params + len(out_names))
    out_specs = (PartitionSpec("core"),) * len(out_names)
    fn = jax.jit(
        shard_map(
            _body, mesh=mesh, in_specs=in_specs, out_specs=out_specs, check_rep=False
        ),
        keep_unused=True,
    )

    per_core = [[np.asarray(m[name]) for name in in_names] for m in in_maps]
    concat_in = [
        np.concatenate([per_core[c][i] for c in range(n_cores)], axis=0)
        for i in range(n_params)
    ]
    concat_zeros = [
        np.zeros((n_cores * z.shape[0], *z.shape[1:]), z.dtype) for z in zero_outs
    ]
    sharding = jax.sharding.NamedSharding(mesh, PartitionSpec("core"))
    args = [jax.device_put(a, sharding) for a in concat_in + concat_zeros]
    return fn, args, out_names, out_avals


def kernel(embeddings, w_att, w_pred, b_pred, **run_kwargs):
    embeddings = prep_emb(embeddings)
    wcat = make_wcat(w_att, w_pred, b_pred)

    nc = _get_nc(PER_CORE)
    in_maps = [
        {
            "emb": embeddings[i * PER_CORE : (i + 1) * PER_CORE],
            "wcat": wcat,
        }
        for i in range(N_CORES)
    ]
    res = run_bass_kernel_spmd(nc, in_maps, core_ids=list(range(N_CORES)), **run_kwargs)
    outs = [res.results[i]["out"].reshape(-1)[:PER_CORE] for i in range(N_CORES)]
    full = np.concatenate(outs).astype(np.float32)
    if run_kwargs:
        return full, res
    return full
